# revision 9
# baseline (speedup 1.0000x reference)
"""LocalContrastEnhancement v7: minimize total engine-busy under the chip
activity throttle.

out = (x - mean) / (sqrt(max(var,1e-6)) + 1e-6), 15x15 zero-padded box.
1 image (3,1024,1024) per NeuronCore, 9 stripes of <=114 output rows.

The TRN2 activity throttle caps sustained multi-engine utilization at
~2 engine-equivalents (measured span ~= total-engine-busy / 2), so the
kernel minimizes summed engine time rather than balancing engines:
  xb f32  = [15 zeros | x (DMA) | 7 zeros]
  sqb f16 = [15x0.25 | (x-0.5)^2 | 7x0.25]      [ACT Square, fp16 out]
  o1/o2   = windowed 15-sum scans, fp16         [DVE; 2.2 cyc/elem floor]
  PD      = -S1~ (fp16 band mm) += 225*x        [bf16 identity mm whose
            moving operand is the f32 buffer bitcast to bf16 hi-halves]
  s1sq    = Square(-PD + corr) fp16             [ACT, mid-group PSUM read]
  P2      = 225*S2~ - s1sq                      [fp16 band mm + negI mm]
  R       = AbsRsqrt(P2 + corr') fp16           [ACT]
  out     = (PD + d_scal) * R                   [DVE scalar_tensor_tensor]
The bf16 truncation's mean bias (E[x-trunc(x)] = 1/768) is folded into
d_scal.  Emission is software-pipelined (sq two stripes ahead, scans one
ahead) and a prologue matmul burst warms the PE HAM clock gate.
Inputs alternate the sync/scalar HWDGE rings; outputs take the other.
"""

import numpy as np
import ml_dtypes

C, H, W = 3, 1024, 1024
NCORES = 8
KS = 15
HALF = 7
MSTR = 114
NHALF = 512

XD0 = 15                    # x data start in xb
XD1 = XD0 + W               # 1039; 7 zero pads follow
XBW = XD1 + 7               # 1046 xb width
SQ0 = 15                    # sq data start in sqb (15 pads of 0.25)
SQ1 = SQ0 + W               # 1039; 7 quarter pads follow
SQW = SQ1 + 7               # 1046 sqb width
SCN1 = W + HALF             # 1031 per-scan output length
O1OFS = HALF                # o1 index of x-box col j is j+7
O2OFS = SCN1 + HALF         # 1038

_CACHE = {}


def _stripes():
    out = []
    r_out = 0
    while r_out < H:
        m = min(MSTR, H - r_out)
        r_in0 = max(r_out - HALF, 0)
        r_in1 = min(r_out + m - 1 + HALF, H - 1)
        k = r_in1 - r_in0 + 1
        k_ofs = HALF - (r_out - r_in0)
        out.append((r_in0, k, r_out, m, k_ofs))
        r_out += m
    return out


def _const_mats():
    band = np.zeros((128, MSTR), dtype=np.float32)
    iden = np.zeros((128, MSTR), dtype=np.float32)
    for m in range(MSTR):
        band[m : m + KS, m] = 1.0
        iden[m + HALF, m] = 225.0
    band_top = np.zeros_like(band)
    band_top[0:121, :] = band[7:128, :]
    iden_top = np.zeros_like(iden)
    iden_top[0:121, :] = iden[7:128, :]
    negi = np.zeros((128, MSTR), dtype=np.float32)
    for m in range(MSTR):
        negi[m, m] = -1.0
    mats16 = [-band, 225.0 * band, -band_top, 225.0 * band_top, negi]
    bands = np.stack(mats16, axis=1).astype(np.float16)  # [128, 5, 114]
    idens = np.stack([iden, iden_top], axis=1).astype(ml_dtypes.bfloat16)

    m_idx = np.arange(128)
    n_top = np.maximum(0, HALF - m_idx).astype(np.float32)
    n_bot = np.maximum(0, m_idx - 104).astype(np.float32)  # bottom stripe M=112
    trunc_comp = 225.0 / 768.0  # mean bf16-truncation bias of 225*x
    corr = np.zeros((128, 3, 3), dtype=np.float32)
    corr[:, 2, 0] = 0.0
    corr[:, 2, 1] = 0.0
    corr[:, 2, 2] = -112.5 + trunc_comp
    for v, n in ((0, n_top), (1, n_bot)):
        corr[:, v, 0] = -7.5 * n
        corr[:, v, 1] = 843.75 * n
        corr[:, v, 2] = 7.5 * n - 112.5 + trunc_comp
    return bands, idens, corr


def _build_nc():
    import concourse.bass as bass
    import concourse.bacc as bacc
    import concourse.tile as tile
    from concourse import mybir
    import bass_rust as _bass_rust
    from concourse.hw_specs import get_activation_tables

    f32 = mybir.dt.float32
    fp16 = mybir.dt.float16
    bf16 = mybir.dt.bfloat16
    Alu = mybir.AluOpType
    Act = mybir.ActivationFunctionType

    class _LceBacc(bacc.Bacc):
        """Pin ACT tables to the single set holding Square+AbsRsqrt."""

        def insert_act_table_loads(self):
            tables = [
                (name, funcs if name == "abs_reciprocal_sqrt_and_small" else set())
                for name, funcs in get_activation_tables(self.m.arch).items()
            ]
            _bass_rust.insert_act_table_loads(self, tables)

    nc = _LceBacc(trn_type="TRN2", target_bir_lowering=False)
    x_d = nc.dram_tensor("x", [C, H, W], f32, kind="ExternalInput")
    bands_d = nc.dram_tensor("bands", [128, 5, MSTR], fp16, kind="ExternalInput")
    idens_d = nc.dram_tensor("idens", [128, 2, MSTR], bf16, kind="ExternalInput")
    corr_d = nc.dram_tensor("corr", [128, 3, 3], f32, kind="ExternalInput")
    y_d = nc.dram_tensor("y", [C, H, W], f32, kind="ExternalOutput")

    stripes = _stripes()
    iters = []
    for c in range(C):
        for r_in0, K, r_out0, M, k_ofs in stripes:
            iters.append((c, r_in0, K, r_out0, M, k_ofs))
    NST = len(iters)

    from contextlib import ExitStack

    with tile.TileContext(nc) as tc, ExitStack() as ctx:
        singles = ctx.enter_context(tc.tile_pool(name="singles", bufs=1))
        io_pool = ctx.enter_context(tc.tile_pool(name="io", bufs=1))
        s1sq_p = ctx.enter_context(tc.tile_pool(name="s1sq", bufs=4))
        r_p = ctx.enter_context(tc.tile_pool(name="rts", bufs=4))
        out_p = ctx.enter_context(tc.tile_pool(name="outb", bufs=4))
        psd_p = ctx.enter_context(tc.tile_pool(name="psd", bufs=2, space="PSUM"))
        ps2_p = ctx.enter_context(tc.tile_pool(name="ps2", bufs=2, space="PSUM"))

        bands_t = singles.tile([128, 5, MSTR], fp16)
        idens_t = singles.tile([128, 2, MSTR], bf16)
        corr_t = singles.tile([128, 3, 3], f32)
        nc.sync.dma_start(out=bands_t[:, :, :], in_=bands_d[:, :, :])
        nc.sync.dma_start(out=idens_t[:, :, :], in_=idens_d[:, :, :])
        nc.sync.dma_start(out=corr_t[:, :, :], in_=corr_d[:, :, :])

        NBUF = 8
        xb = [io_pool.tile([128, XBW], f32, tag=f"xb{i}", name=f"xb{i}") for i in range(NBUF)]
        sb = [io_pool.tile([128, SQW], fp16, tag=f"sb{i}", name=f"sb{i}") for i in range(NBUF)]
        ob = [io_pool.tile([128, 2 * SCN1], fp16, tag=f"ob{i}", name=f"ob{i}") for i in range(NBUF)]
        for i in range(NBUF):
            nc.vector.memset(xb[i][:, 0:XD0], 0.0)
            nc.vector.memset(xb[i][:, XD1:XBW], 0.0)
            nc.vector.memset(sb[i][:, 0:SQ0], 0.25)
            nc.vector.memset(sb[i][:, SQ1:SQW], 0.25)

        neghalf = singles.tile([128, 1], f32)
        nc.vector.memset(neghalf[:, :], -0.5)
        warm1 = singles.tile([128, 1], f32)
        warm2 = singles.tile([128, 1], f32)
        warm3 = singles.tile([128, 1], f32)
        nc.scalar.activation(out=warm1[:, :], in_=corr_t[:, 0, 0:1], func=Act.Square)
        nc.scalar.activation(out=warm2[:, :], in_=neghalf[:, :], func=Act.Square)
        nc.scalar.activation(
            out=warm3[:, :], in_=warm2[:, :], func=Act.Abs_reciprocal_sqrt
        )
        # PE warm-up burst: flip the HAM clock gate to 8/8 before stripe 0.
        pwarm = psd_p.tile([MSTR, W], f32, tag="pd")
        for _ in range(28):
            nc.tensor.matmul(
                pwarm[0:MSTR, 0:MSTR],
                bands_t[0:128, 1, 0:MSTR],
                bands_t[0:128, 0, 0:MSTR],
                start=True,
                stop=True,
            )
        warm4 = singles.tile([128, 1], f32)
        nc.scalar.activation(out=warm4[0:1, :], in_=pwarm[0:1, 0:1], func=Act.Copy)

        def dma_in(it):
            c, r_in0, K, r_out0, M, k_ofs = iters[it]
            eng = nc.sync if it % 2 == 0 else nc.scalar
            eng.dma_start(
                out=xb[it % NBUF][0:K, XD0:XD1],
                in_=x_d[c, r_in0 : r_in0 + K, :],
            )

        def sq_act(it):
            c, r_in0, K, r_out0, M, k_ofs = iters[it]
            nc.scalar.activation(
                out=sb[it % NBUF][0:K, SQ0:SQ1],
                in_=xb[it % NBUF][0:K, XD0:XD1],
                func=Act.Square,
                bias=neghalf[0:K, 0:1],
            )

        def xscan(it):
            c, r_in0, K, r_out0, M, k_ofs = iters[it]
            xt, ot = xb[it % NBUF], ob[it % NBUF]
            nc.vector.tensor_tensor_scan(
                out=ot[0:K, 0:SCN1],
                data0=xt[0:K, KS : KS + SCN1],
                data1=xt[0:K, 0:SCN1],
                initial=-7.5,
                op0=Alu.add,
                op1=Alu.subtract,
            )

        def sqscan(it):
            c, r_in0, K, r_out0, M, k_ofs = iters[it]
            st, ot = sb[it % NBUF], ob[it % NBUF]
            nc.vector.tensor_tensor_scan(
                out=ot[0:K, SCN1 : 2 * SCN1],
                data0=st[0:K, KS : KS + SCN1],
                data1=st[0:K, 0:SCN1],
                initial=3.75,
                op0=Alu.add,
                op1=Alu.subtract,
            )

        back_state = {}

        def back_head(it):
            """phase1 matmuls + s1sq for stripe it"""
            c, r_in0, K, r_out0, M, k_ofs = iters[it]
            ot = ob[it % NBUF]
            bofs = 2 if k_ofs else 0
            vv = 0 if k_ofs else (1 if r_out0 + M == H else 2)
            sq_bias = corr_t[0:M, vv, 0:1]

            pd = psd_p.tile([MSTR, W], f32, tag="pd")
            p2 = ps2_p.tile([MSTR, W], f32)
            for j0 in (0, NHALF):
                nc.tensor.matmul(
                    pd[0:M, j0 : j0 + NHALF],
                    bands_t[0:K, bofs, 0:M],
                    ot[0:K, O1OFS + j0 : O1OFS + j0 + NHALF],
                    start=True,
                    stop=False,
                )
            s1sq = s1sq_p.tile([MSTR, W], fp16)
            nc.scalar.activation(
                out=s1sq[0:M, :],
                in_=pd[0:M, :],
                func=Act.Square,
                scale=-1.0,
                bias=sq_bias,
            )
            back_state[it] = (pd, p2, s1sq)

        def back_tail(it):
            """phase2/P2 matmuls, rsqrt, final STT, out-dma for stripe it"""
            c, r_in0, K, r_out0, M, k_ofs = iters[it]
            xt, ot = xb[it % NBUF], ob[it % NBUF]
            bofs = 2 if k_ofs else 0
            isel = 1 if k_ofs else 0
            vv = 0 if k_ofs else (1 if r_out0 + M == H else 2)
            p2_bias = corr_t[0:M, vv, 1:2]
            d_scal = corr_t[0:M, vv, 2:3]
            pd, p2, s1sq = back_state.pop(it)
            xbf = xt[0:K, :].bitcast(bf16)  # [K, 2*XBW]; odd cols = hi halves
            for j0 in (0, NHALF):
                nc.tensor.matmul(
                    pd[0:M, j0 : j0 + NHALF],
                    idens_t[0:K, isel, 0:M],
                    xbf[0:K, 2 * (XD0 + j0) + 1 : 2 * (XD0 + j0 + NHALF) : 2],
                    start=False,
                    stop=True,
                    skip_group_check=True,
                )
                nc.tensor.matmul(
                    p2[0:M, j0 : j0 + NHALF],
                    bands_t[0:K, bofs + 1, 0:M],
                    ot[0:K, O2OFS + j0 : O2OFS + j0 + NHALF],
                    start=True,
                    stop=False,
                )
            for j0 in (0, NHALF):
                nc.tensor.matmul(
                    p2[0:M, j0 : j0 + NHALF],
                    bands_t[0:M, 4, 0:M],
                    s1sq[0:M, j0 : j0 + NHALF],
                    start=False,
                    stop=True,
                )
            rts = r_p.tile([MSTR, W], fp16)
            nc.scalar.activation(
                out=rts[0:M, :],
                in_=p2[0:M, :],
                func=Act.Abs_reciprocal_sqrt,
                bias=p2_bias,
            )
            outb = out_p.tile([MSTR, W], f32)
            nc.vector.scalar_tensor_tensor(
                out=outb[0:M, :],
                in0=pd[0:M, :],
                scalar=d_scal,
                in1=rts[0:M, :],
                op0=Alu.add,
                op1=Alu.mult,
            )
            eng = nc.scalar if it % 2 == 0 else nc.sync
            eng.dma_start(out=y_d[c, r_out0 : r_out0 + M, :], in_=outb[0:M, :])

        # skewed pipeline
        for j in range(4):
            dma_in(j)
        sq_act(0)
        sq_act(1)
        sq_act(2)
        xscan(0)
        xscan(1)
        sqscan(0)
        for it in range(NST):
            back_head(it)
            if it + 4 < NST:
                dma_in(it + 4)
                # HAM keep-alive fillers: tiny matmuls gated on mid-period
                # events, into p2 regions the next start=True mm re-clears.
                p2f = back_state[it][1]
                nc.tensor.matmul(
                    p2f[0:MSTR, NHALF : NHALF + 128],
                    bands_t[0:128, 1, 0:MSTR],
                    xb[(it + 3) % NBUF][0:128, XD0 : XD0 + 128].bitcast(bf16)[
                        0:128, 1:256:2
                    ],
                    start=True,
                    stop=True,
                )
            if it + 3 < NST:
                sq_act(it + 3)
            if it + 2 < NST:
                p2f = back_state[it][1]
                nc.tensor.matmul(
                    p2f[0:MSTR, 0:128],
                    bands_t[0:128, 1, 0:MSTR],
                    sb[(it + 2) % NBUF][0:128, SQ0 : SQ0 + 128],
                    start=True,
                    stop=True,
                )
            if it + 1 < NST:
                sqscan(it + 1)
            if it + 2 < NST:
                xscan(it + 2)
            back_tail(it)

    nc.finalize()
    return nc


def _get_nc():
    if "nc" not in _CACHE:
        _CACHE["nc"] = _build_nc()
    return _CACHE["nc"]


def kernel(x: np.ndarray, _trace: bool = False, _tmpdir=None) -> np.ndarray:
    from concourse.bass_utils import run_bass_kernel_spmd

    assert x.shape == (NCORES, C, H, W), x.shape
    nc = _get_nc()
    bands, idens, corr = _const_mats()
    in_maps = [
        {
            "x": np.ascontiguousarray(x[i]).astype(np.float32, copy=False),
            "bands": bands,
            "idens": idens,
            "corr": corr,
        }
        for i in range(NCORES)
    ]
    res = run_bass_kernel_spmd(
        nc,
        in_maps,
        core_ids=list(range(NCORES)),
        trace=_trace,
        tmpdir=_tmpdir,
    )
    _CACHE["last_results"] = res
    out = np.stack([r["y"] for r in res.results], axis=0)
    return out


if __name__ == "__main__":
    rng = np.random.default_rng(0)
    x = rng.random((NCORES, C, H, W), dtype=np.float32)
    y = kernel(x)
    print(y.shape, y.dtype, float(np.abs(y).mean()))


# revision 10
# speedup vs baseline: 1.0244x; 1.0244x over previous
"""LocalContrastEnhancement v7: minimize total engine-busy under the chip
activity throttle.

out = (x - mean) / (sqrt(max(var,1e-6)) + 1e-6), 15x15 zero-padded box.
1 image (3,1024,1024) per NeuronCore, 9 stripes of <=114 output rows.

The TRN2 activity throttle caps sustained multi-engine utilization at
~2 engine-equivalents (measured span ~= total-engine-busy / 2), so the
kernel minimizes summed engine time rather than balancing engines:
  xb f32  = [15 zeros | x (DMA) | 7 zeros]
  sqb f16 = [15x0.25 | (x-0.5)^2 | 7x0.25]      [ACT Square, fp16 out]
  o1/o2   = windowed 15-sum scans, fp16         [DVE; 2.2 cyc/elem floor]
  PD      = -S1~ (fp16 band mm) += 225*x        [bf16 identity mm whose
            moving operand is the f32 buffer bitcast to bf16 hi-halves]
  s1sq    = Square(-PD + corr) fp16             [ACT, mid-group PSUM read]
  P2      = 225*S2~ - s1sq                      [fp16 band mm + negI mm]
  R       = AbsRsqrt(P2 + corr') fp16           [ACT]
  out     = (PD + d_scal) * R                   [DVE scalar_tensor_tensor]
The bf16 truncation's mean bias (E[x-trunc(x)] = 1/768) is folded into
d_scal.  Emission is software-pipelined (sq two stripes ahead, scans one
ahead) and a prologue matmul burst warms the PE HAM clock gate.
Inputs alternate the sync/scalar HWDGE rings; outputs take the other.
"""

import numpy as np
import ml_dtypes

C, H, W = 3, 1024, 1024
NCORES = 8
KS = 15
HALF = 7
MSTR = 114
NHALF = 512

XD0 = 15                    # x data start in xb
XD1 = XD0 + W               # 1039; 7 zero pads follow
XBW = XD1 + 7               # 1046 xb width
SQ0 = 15                    # sq data start in sqb (15 pads of 0.25)
SQ1 = SQ0 + W               # 1039; 7 quarter pads follow
SQW = SQ1 + 7               # 1046 sqb width
SCN1 = W + HALF             # 1031 per-scan output length
O1OFS = HALF                # o1 index of x-box col j is j+7
O2OFS = SCN1 + HALF         # 1038

_CACHE = {}


def _stripes():
    out = []
    r_out = 0
    while r_out < H:
        m = min(MSTR, H - r_out)
        r_in0 = max(r_out - HALF, 0)
        r_in1 = min(r_out + m - 1 + HALF, H - 1)
        k = r_in1 - r_in0 + 1
        k_ofs = HALF - (r_out - r_in0)
        out.append((r_in0, k, r_out, m, k_ofs))
        r_out += m
    return out


def _const_mats():
    band = np.zeros((128, MSTR), dtype=np.float32)
    iden = np.zeros((128, MSTR), dtype=np.float32)
    for m in range(MSTR):
        band[m : m + KS, m] = 1.0
        iden[m + HALF, m] = 225.0
    band_top = np.zeros_like(band)
    band_top[0:121, :] = band[7:128, :]
    iden_top = np.zeros_like(iden)
    iden_top[0:121, :] = iden[7:128, :]
    negi = np.zeros((128, MSTR), dtype=np.float32)
    for m in range(MSTR):
        negi[m, m] = -1.0
    mats16 = [-band, 225.0 * band, -band_top, 225.0 * band_top, negi]
    bands = np.stack(mats16, axis=1).astype(np.float16)  # [128, 5, 114]
    idens = np.stack([iden, iden_top], axis=1).astype(ml_dtypes.bfloat16)

    m_idx = np.arange(128)
    n_top = np.maximum(0, HALF - m_idx).astype(np.float32)
    n_bot = np.maximum(0, m_idx - 104).astype(np.float32)  # bottom stripe M=112
    trunc_comp = 225.0 / 768.0  # mean bf16-truncation bias of 225*x
    corr = np.zeros((128, 3, 3), dtype=np.float32)
    corr[:, 2, 0] = 0.0
    corr[:, 2, 1] = 0.0
    corr[:, 2, 2] = -112.5 + trunc_comp
    for v, n in ((0, n_top), (1, n_bot)):
        corr[:, v, 0] = -7.5 * n
        corr[:, v, 1] = 843.75 * n
        corr[:, v, 2] = 7.5 * n - 112.5 + trunc_comp
    return bands, idens, corr


def _build_nc():
    import concourse.bass as bass
    import concourse.bacc as bacc
    import concourse.tile as tile
    from concourse import mybir
    import bass_rust as _bass_rust
    from concourse.hw_specs import get_activation_tables

    f32 = mybir.dt.float32
    fp16 = mybir.dt.float16
    bf16 = mybir.dt.bfloat16
    Alu = mybir.AluOpType
    Act = mybir.ActivationFunctionType

    class _LceBacc(bacc.Bacc):
        """Pin ACT tables to the single set holding Square+AbsRsqrt."""

        def insert_act_table_loads(self):
            tables = [
                (name, funcs if name == "abs_reciprocal_sqrt_and_small" else set())
                for name, funcs in get_activation_tables(self.m.arch).items()
            ]
            _bass_rust.insert_act_table_loads(self, tables)

    nc = _LceBacc(trn_type="TRN2", target_bir_lowering=False)
    x_d = nc.dram_tensor("x", [C, H, W], f32, kind="ExternalInput")
    bands_d = nc.dram_tensor("bands", [128, 5, MSTR], fp16, kind="ExternalInput")
    idens_d = nc.dram_tensor("idens", [128, 2, MSTR], bf16, kind="ExternalInput")
    corr_d = nc.dram_tensor("corr", [128, 3, 3], f32, kind="ExternalInput")
    y_d = nc.dram_tensor("y", [C, H, W], f32, kind="ExternalOutput")

    stripes = _stripes()
    iters = []
    for c in range(C):
        for r_in0, K, r_out0, M, k_ofs in stripes:
            iters.append((c, r_in0, K, r_out0, M, k_ofs))
    NST = len(iters)

    from contextlib import ExitStack

    with tile.TileContext(nc) as tc, ExitStack() as ctx:
        singles = ctx.enter_context(tc.tile_pool(name="singles", bufs=1))
        io_pool = ctx.enter_context(tc.tile_pool(name="io", bufs=1))
        s1sq_p = ctx.enter_context(tc.tile_pool(name="s1sq", bufs=4))
        r_p = ctx.enter_context(tc.tile_pool(name="rts", bufs=4))
        out_p = ctx.enter_context(tc.tile_pool(name="outb", bufs=4))
        psd_p = ctx.enter_context(tc.tile_pool(name="psd", bufs=2, space="PSUM"))
        ps2_p = ctx.enter_context(tc.tile_pool(name="ps2", bufs=2, space="PSUM"))

        bands_t = singles.tile([128, 5, MSTR], fp16)
        idens_t = singles.tile([128, 2, MSTR], bf16)
        corr_t = singles.tile([128, 3, 3], f32)
        nc.sync.dma_start(out=bands_t[:, :, :], in_=bands_d[:, :, :])
        nc.sync.dma_start(out=idens_t[:, :, :], in_=idens_d[:, :, :])
        nc.sync.dma_start(out=corr_t[:, :, :], in_=corr_d[:, :, :])

        NBUF = 7
        xb = [io_pool.tile([128, XBW], f32, tag=f"xb{i}", name=f"xb{i}") for i in range(NBUF)]
        sb = [io_pool.tile([128, SQW], fp16, tag=f"sb{i}", name=f"sb{i}") for i in range(NBUF)]
        ob = [io_pool.tile([128, 2 * SCN1], fp16, tag=f"ob{i}", name=f"ob{i}") for i in range(NBUF)]
        for i in range(NBUF):
            nc.vector.memset(xb[i][:, 0:XD0], 0.0)
            nc.vector.memset(xb[i][:, XD1:XBW], 0.0)
            nc.vector.memset(sb[i][:, 0:SQ0], 0.25)
            nc.vector.memset(sb[i][:, SQ1:SQW], 0.25)

        neghalf = singles.tile([128, 1], f32)
        nc.vector.memset(neghalf[:, :], -0.5)
        warm1 = singles.tile([128, 1], f32)
        warm2 = singles.tile([128, 1], f32)
        warm3 = singles.tile([128, 1], f32)
        nc.scalar.activation(out=warm1[:, :], in_=corr_t[:, 0, 0:1], func=Act.Square)
        nc.scalar.activation(out=warm2[:, :], in_=neghalf[:, :], func=Act.Square)
        nc.scalar.activation(
            out=warm3[:, :], in_=warm2[:, :], func=Act.Abs_reciprocal_sqrt
        )
        # PE warm-up burst: flip the HAM clock gate to 8/8 before stripe 0.
        pwarm = psd_p.tile([MSTR, W], f32, tag="pd")
        for _ in range(28):
            nc.tensor.matmul(
                pwarm[0:MSTR, 0:MSTR],
                bands_t[0:128, 1, 0:MSTR],
                bands_t[0:128, 0, 0:MSTR],
                start=True,
                stop=True,
            )
        warm4 = singles.tile([128, 1], f32)
        nc.scalar.activation(out=warm4[0:1, :], in_=pwarm[0:1, 0:1], func=Act.Copy)

        def dma_in(it):
            c, r_in0, K, r_out0, M, k_ofs = iters[it]
            eng = nc.sync if it % 2 == 0 else nc.scalar
            eng.dma_start(
                out=xb[it % NBUF][0:K, XD0:XD1],
                in_=x_d[c, r_in0 : r_in0 + K, :],
            )

        def sq_act(it):
            c, r_in0, K, r_out0, M, k_ofs = iters[it]
            nc.scalar.activation(
                out=sb[it % NBUF][0:K, SQ0:SQ1],
                in_=xb[it % NBUF][0:K, XD0:XD1],
                func=Act.Square,
                bias=neghalf[0:K, 0:1],
            )

        def xscan(it):
            c, r_in0, K, r_out0, M, k_ofs = iters[it]
            xt, ot = xb[it % NBUF], ob[it % NBUF]
            nc.vector.tensor_tensor_scan(
                out=ot[0:K, 0:SCN1],
                data0=xt[0:K, KS : KS + SCN1],
                data1=xt[0:K, 0:SCN1],
                initial=-7.5,
                op0=Alu.add,
                op1=Alu.subtract,
            )

        def sqscan(it):
            c, r_in0, K, r_out0, M, k_ofs = iters[it]
            st, ot = sb[it % NBUF], ob[it % NBUF]
            nc.vector.tensor_tensor_scan(
                out=ot[0:K, SCN1 : 2 * SCN1],
                data0=st[0:K, KS : KS + SCN1],
                data1=st[0:K, 0:SCN1],
                initial=3.75,
                op0=Alu.add,
                op1=Alu.subtract,
            )

        back_state = {}

        def back_head(it):
            """phase1 matmuls + s1sq for stripe it"""
            c, r_in0, K, r_out0, M, k_ofs = iters[it]
            ot = ob[it % NBUF]
            bofs = 2 if k_ofs else 0
            vv = 0 if k_ofs else (1 if r_out0 + M == H else 2)
            sq_bias = corr_t[0:M, vv, 0:1]

            pd = psd_p.tile([MSTR, W], f32, tag="pd")
            p2 = ps2_p.tile([MSTR, W], f32)
            for j0 in (0, NHALF):
                nc.tensor.matmul(
                    pd[0:M, j0 : j0 + NHALF],
                    bands_t[0:K, bofs, 0:M],
                    ot[0:K, O1OFS + j0 : O1OFS + j0 + NHALF],
                    start=True,
                    stop=False,
                )
            s1sq = s1sq_p.tile([MSTR, W], fp16)
            nc.scalar.activation(
                out=s1sq[0:M, :],
                in_=pd[0:M, :],
                func=Act.Square,
                scale=-1.0,
                bias=sq_bias,
            )
            back_state[it] = (pd, p2, s1sq)

        def back_tail(it):
            """phase2/P2 matmuls, rsqrt, final STT, out-dma for stripe it"""
            c, r_in0, K, r_out0, M, k_ofs = iters[it]
            xt, ot = xb[it % NBUF], ob[it % NBUF]
            bofs = 2 if k_ofs else 0
            isel = 1 if k_ofs else 0
            vv = 0 if k_ofs else (1 if r_out0 + M == H else 2)
            p2_bias = corr_t[0:M, vv, 1:2]
            d_scal = corr_t[0:M, vv, 2:3]
            pd, p2, s1sq = back_state.pop(it)
            xbf = xt[0:K, :].bitcast(bf16)  # [K, 2*XBW]; odd cols = hi halves
            for j0 in (0, NHALF):
                nc.tensor.matmul(
                    pd[0:M, j0 : j0 + NHALF],
                    idens_t[0:K, isel, 0:M],
                    xbf[0:K, 2 * (XD0 + j0) + 1 : 2 * (XD0 + j0 + NHALF) : 2],
                    start=False,
                    stop=True,
                    skip_group_check=True,
                )
                nc.tensor.matmul(
                    p2[0:M, j0 : j0 + NHALF],
                    bands_t[0:K, bofs + 1, 0:M],
                    ot[0:K, O2OFS + j0 : O2OFS + j0 + NHALF],
                    start=True,
                    stop=False,
                )
            for j0 in (0, NHALF):
                nc.tensor.matmul(
                    p2[0:M, j0 : j0 + NHALF],
                    bands_t[0:M, 4, 0:M],
                    s1sq[0:M, j0 : j0 + NHALF],
                    start=False,
                    stop=True,
                )
            rts = r_p.tile([MSTR, W], fp16)
            nc.scalar.activation(
                out=rts[0:M, :],
                in_=p2[0:M, :],
                func=Act.Abs_reciprocal_sqrt,
                bias=p2_bias,
            )
            outb = out_p.tile([MSTR, W], f32)
            nc.vector.scalar_tensor_tensor(
                out=outb[0:M, :],
                in0=pd[0:M, :],
                scalar=d_scal,
                in1=rts[0:M, :],
                op0=Alu.add,
                op1=Alu.mult,
            )
            eng = nc.scalar if it % 2 == 0 else nc.sync
            eng.dma_start(out=y_d[c, r_out0 : r_out0 + M, :], in_=outb[0:M, :])

        # skewed pipeline
        dma_in(0)
        dma_in(1)
        dma_in(2)
        sq_act(0)
        sq_act(1)
        xscan(0)
        xscan(1)
        sqscan(0)
        for it in range(NST):
            if it + 3 < NST:
                dma_in(it + 3)
            back_head(it)
            if it + 2 < NST:
                sq_act(it + 2)
                # HAM keep-alive: a tiny matmul gated on the mid-period
                # square, into a p2 region the next start=True mm re-clears.
                p2f = back_state[it][1]
                nc.tensor.matmul(
                    p2f[0:MSTR, 0:128],
                    bands_t[0:128, 1, 0:MSTR],
                    sb[(it + 2) % NBUF][0:128, SQ0 : SQ0 + 128],
                    start=True,
                    stop=True,
                )
            if it + 1 < NST:
                sqscan(it + 1)
            if it + 2 < NST:
                xscan(it + 2)
            back_tail(it)

    nc.finalize()
    return nc


def _get_nc():
    if "nc" not in _CACHE:
        _CACHE["nc"] = _build_nc()
    return _CACHE["nc"]


def kernel(x: np.ndarray, _trace: bool = False, _tmpdir=None) -> np.ndarray:
    from concourse.bass_utils import run_bass_kernel_spmd

    assert x.shape == (NCORES, C, H, W), x.shape
    nc = _get_nc()
    bands, idens, corr = _const_mats()
    in_maps = [
        {
            "x": np.ascontiguousarray(x[i]).astype(np.float32, copy=False),
            "bands": bands,
            "idens": idens,
            "corr": corr,
        }
        for i in range(NCORES)
    ]
    res = run_bass_kernel_spmd(
        nc,
        in_maps,
        core_ids=list(range(NCORES)),
        trace=_trace,
        tmpdir=_tmpdir,
    )
    _CACHE["last_results"] = res
    out = np.stack([r["y"] for r in res.results], axis=0)
    return out


if __name__ == "__main__":
    rng = np.random.default_rng(0)
    x = rng.random((NCORES, C, H, W), dtype=np.float32)
    y = kernel(x)
    print(y.shape, y.dtype, float(np.abs(y).mean()))
